# revision 9
# baseline (speedup 1.0000x reference)
"""Trainium2 Bass kernel for nn_Conv2d_71476845740806.

Reference semantics (buggy naive Conv2d):
  xsum = pad(input, 1).sum(batch)                  # (1, C, 258, 258)
  conv = conv2d(xsum, weight, stride=2, VALID)     # (1, K, 128, 128)
  vals = conv[0, :, :64, :64]                      # (K, 64, 64)
  out  = zeros(B, K, 128, 128); out[:, :, ::2, ::2] = vals  (batch-replicated)

Only window starts (2i, 2j), i,j in [0,64) are used -> only padded rows/cols
0..128 of the summed image matter -> only input rows/cols 0..127.

Device strategy (8 cores, SPMD), v2:
  - Shard the 64 output rows: core q computes rows 8q..8q+7 for ALL K=128
    filters. Needs padded rows 16q..16q+16 (17 rows) x 129 cols, all b, c.
  - bf16 wire format end to end (tolerance is 2e-2; bf16 costs ~0.3% rel
    err): halves HBM traffic and doubles PE/DVE throughput vs fp32/f32r.
  - Host packs per-core xc[128, 8, 1170] bf16:
    partitions 0..63 = (c, even padded rows 0..8), 64..127 = (c, odd rows).
    Each 130-col row is stored even/odd-column-split: [65 even | 65 odd],
    so every conv matmul reads a UNIT-STRIDE 64-col run (the baseline's
    stride-2 rhs read halved PE throughput).
  - Raw hand-synchronized program (no TileContext): the tile epilogue's
    semaphore-reset storm costs ~8us on its own.
  - Batch sum: 4 DVE pair adds as chunks land + 2 quad adds; conv as
    2 groups x 6 matmuls (3 contract-128 for kernel rows 0/1 paired
    across partition halves, 3 contract-64 for row 2) accumulating in
    one PSUM bank [128, 512].
  - Weights ride the scalar-engine HWDGE queue in parallel with input
    chunks on the sync-engine queue; DVE copies PSUM->SBUF; sync DMAs
    the per-core vals (128, 512) out. Host scatters into the zero output.
"""

import contextlib

import ml_dtypes
import numpy as np

import concourse.bacc as bacc
import concourse.mybir as mybir
from concourse.bass_utils import run_bass_kernel_spmd

F32 = mybir.dt.float32
BF16 = mybir.dt.bfloat16

B, C, H, W = 8, 64, 256, 256
K = 128
NCORES = 8
RPC = 8                    # output rows per core (64 total)
ER = 9                     # even padded rows per core
OR = 8                     # odd padded rows per core
WC = 130                   # stored cols per row: [65 even | 65 odd]
PITCH = ER * WC            # 1170 per-batch free pitch
NOUT = RPC * 64            # 512
COLOFF = (0, 65, 1)        # col block offset by kernel-dw (even j / odd j / even j+1)

# Input DMA chunks (must divide 8): 2 = 4-batch chunks (9.4KB bf16 runs),
# 4 = 2-batch chunks (4.7KB runs), 8 = single batches.
NCHUNK = 2
# Alternate input chunks across the two HWDGE queues (sync + scalar).
SPLITQ = False
# DMA the output as bf16 (host upcasts); halves the tail DMA.
OUT_BF16 = True
# PE p-state warmup: the PE reaches its full 2.4 GHz clock only after ~3us
# of continuous execution (else it runs at 0.65-1.2 GHz). Dummy matmuls on
# (uninitialized) SBUF into a scratch PSUM bank keep it busy from t=0.
WARM1 = 14                 # dummies before the first real group
WARM2 = 6                  # dummies between the two real groups
# Skip the final wait on the output-DMA completion semaphore; the engine
# epilogue drain already blocks until the queue is empty, and skipping lets
# the fixed end-of-kernel semaphore-clear ucode overlap the out-DMA.
NOWAIT_ODMA = True

TRACE = False
LAST_EXEC_NS = None

_cache = {}


def _build_program():
    key = (NCHUNK, SPLITQ, OUT_BF16, WARM1, WARM2, NOWAIT_ODMA)
    if key in _cache:
        return _cache[key]

    out_dt = BF16 if OUT_BF16 else F32
    bpc = B // NCHUNK  # batches per DMA chunk

    nc = bacc.Bacc(None)
    xc = nc.declare_dram_parameter("xc", [128, B, PITCH], BF16, isOutput=False)
    # cols 0:384 = pair weights (kernel rows 0/1 on partition halves),
    # cols 384:768 (partitions 0:64) = kernel-row-2 weights.
    wc = nc.declare_dram_parameter("wc", [128, 6 * K], BF16, isOutput=False)
    out = nc.declare_dram_parameter("out", [K, NOUT], out_dt, isOutput=True)

    ctx = contextlib.ExitStack()
    wt = ctx.enter_context(nc.sbuf_tensor([128, 6 * K], BF16))
    staging = ctx.enter_context(nc.sbuf_tensor([128, B * PITCH], BF16))
    spair = ctx.enter_context(nc.sbuf_tensor([128, 4 * PITCH], BF16))
    squad = ctx.enter_context(nc.sbuf_tensor([128, 2 * PITCH], BF16))
    outs = ctx.enter_context(nc.sbuf_tensor([K, NOUT], out_dt))
    psum = ctx.enter_context(nc.psum_tensor([K, NOUT], F32))
    dpsum = ctx.enter_context(nc.psum_tensor([K, NOUT], F32))
    in_semA = ctx.enter_context(nc.semaphore("in_semA"))
    in_semB = ctx.enter_context(nc.semaphore("in_semB"))
    w_sem = ctx.enter_context(nc.semaphore("w_sem"))
    qadd_sem = ctx.enter_context(nc.semaphore("qadd_sem"))
    mm_sem = ctx.enter_context(nc.semaphore("mm_sem"))
    cp_sem = ctx.enter_context(nc.semaphore("cp_sem"))
    odma_sem = ctx.enter_context(nc.semaphore("odma_sem"))

    wpt3 = wt[:, 0 : 3 * K].rearrange("p (a b) -> p a b", a=3)
    w2t3 = wt[:, 3 * K : 6 * K].rearrange("p (a b) -> p a b", a=3)
    st3 = staging[:, :].rearrange("p (b x) -> p b x", b=B)
    sp3 = spair[:, :].rearrange("p (b x) -> p b x", b=4)
    sq3 = squad[:, :].rearrange("p (b x) -> p b x", b=2)
    psum3 = psum[:, :].rearrange("p (r w) -> p r w", r=RPC)
    xcr = xc[:, :, :]

    def chunk_queue(ch):
        return (ch % 2) if SPLITQ else 0

    def waits_for_chunks(n):
        """(semA_count, semB_count) needed for chunks 0..n-1 complete."""
        if not SPLITQ:
            return n * 16, 0
        a = sum(1 for c in range(n) if chunk_queue(c) == 0)
        b = n - a
        return a * 16, b * 16

    with nc.Block(no_gpsimd_drain=True) as block:

        @block.sync
        def _(sync):
            for ch in range(NCHUNK):
                if chunk_queue(ch) != 0:
                    continue
                b0 = ch * bpc
                sync.dma_start(
                    out=st3[:, b0 : b0 + bpc, :], in_=xcr[:, b0 : b0 + bpc, :]
                ).then_inc(in_semA, 16)
            sync.wait_ge(cp_sem, 1)
            sync.dma_start(out=out[:, :], in_=outs[:, :]).then_inc(odma_sem, 16)
            if not NOWAIT_ODMA:
                sync.wait_ge(odma_sem, 16)

        @block.scalar
        def _(sc):
            sc.dma_start(out=wt[:, :], in_=wc[:, :]).then_inc(w_sem, 16)
            if SPLITQ:
                for ch in range(NCHUNK):
                    if chunk_queue(ch) != 1:
                        continue
                    b0 = ch * bpc
                    sc.dma_start(
                        out=st3[:, b0 : b0 + bpc, :], in_=xcr[:, b0 : b0 + bpc, :]
                    ).then_inc(in_semB, 16)

        @block.vector
        def _(v):
            seenA = seenB = 0
            for bp in range(4):
                nchunks = (2 * bp + 1) // bpc + 1  # chunks needed for batches 2bp,2bp+1
                needA, needB = waits_for_chunks(nchunks)
                if needA > seenA:
                    v.wait_ge(in_semA, needA)
                    seenA = needA
                if needB > seenB:
                    v.wait_ge(in_semB, needB)
                    seenB = needB
                v.tensor_add(sp3[:, bp, :], st3[:, 2 * bp, :], st3[:, 2 * bp + 1, :])
                if bp % 2 == 1:
                    h = bp // 2
                    v.tensor_add(
                        sq3[:, h, :], sp3[:, 2 * h, :], sp3[:, 2 * h + 1, :]
                    ).then_inc(qadd_sem, 1)
            v.wait_ge(mm_sem, 1)
            v.tensor_copy(outs[:, :], psum[:, :]).then_inc(cp_sem, 1)

        @block.tensor
        def _(t):
            def dummy_mm():
                nc.tensor.matmul(
                    dpsum[:, :],
                    staging[:, 0:128],
                    staging[:, 128:640],
                    start=True,
                    stop=True,
                    skip_group_check=True,
                )

            for _ in range(WARM1):
                dummy_mm()
            t.wait_ge(w_sem, 16)
            for h in range(2):
                t.wait_ge(qadd_sem, h + 1)
                src3 = sq3[:, h, :].rearrange("p (r w) -> p r w", r=ER)
                for dw in range(3):
                    # kernel rows 0 (even rows, row i) + 1 (odd rows, row i)
                    off = COLOFF[dw]
                    nc.tensor.matmul(
                        psum3[:, :, :],
                        wpt3[:, dw, :],
                        src3[:, 0:RPC, off : off + 64],
                        start=(h == 0 and dw == 0),
                        stop=False,
                    )
                for dw in range(3):
                    # kernel row 2: even rows, row i+1
                    off = COLOFF[dw]
                    nc.tensor.matmul(
                        psum3[0:K, :, :],
                        w2t3[0:64, dw, :],
                        src3[0:64, 1 : 1 + RPC, off : off + 64],
                        start=False,
                        stop=(h == 1 and dw == 2),
                    )
                if h == 0:
                    for _ in range(WARM2):
                        dummy_mm()
            # Flush the PE pipeline so the PSUM accumulation is visible
            # before the DVE copy (matmul retire does not imply PSUM
            # writeback -- racing this was the old raw variant's bug).
            t.drain()
            t.sem_inc(mm_sem, 1)

    nc.compile()
    ctx.close()
    _cache[key] = nc
    return nc


def _prep_inputs(input, weight):
    inp = np.ascontiguousarray(input, dtype=np.float32)
    w = np.ascontiguousarray(weight, dtype=np.float32)

    # Padded top-left region: P[r, w] = padded coord (orig r-1, w-1);
    # only padded rows/cols 0..128 are ever read (row/col 129 stays zero).
    P = np.zeros((B, C, 130, 130), np.float32)
    P[:, :, 1:129, 1:129] = inp[:, :, :128, :128]
    # Even/odd column split: row -> [65 even cols | 65 odd cols], so the
    # device matmuls read unit-stride 64-col runs.
    R = np.concatenate([P[:, :, :, 0:130:2], P[:, :, :, 1:130:2]], axis=3)
    Rc = np.ascontiguousarray(R.transpose(1, 0, 2, 3))  # (C, B, 130, 130)

    t = [w[:, :, dh, :].transpose(1, 2, 0).reshape(-1, 3 * K) for dh in range(3)]
    wc_host = np.zeros((128, 6 * K), np.float32)
    wc_host[0:64, 0 : 3 * K] = t[0]
    wc_host[64:128, 0 : 3 * K] = t[1]
    wc_host[0:64, 3 * K : 6 * K] = t[2]
    wc_host = np.ascontiguousarray(wc_host.astype(ml_dtypes.bfloat16))

    in_maps = []
    for q in range(NCORES):
        r0 = 16 * q
        xcq = np.zeros((128, B, PITCH), np.float32)
        xcq[0:64] = Rc[:, :, r0 : r0 + 17 : 2, :].reshape(64, B, PITCH)
        xcq[64:128, :, 0 : OR * WC] = Rc[:, :, r0 + 1 : r0 + 16 : 2, :].reshape(
            64, B, OR * WC
        )
        in_maps.append(
            {"xc": np.ascontiguousarray(xcq.astype(ml_dtypes.bfloat16)), "wc": wc_host}
        )
    return in_maps


def kernel(input, weight):
    global LAST_EXEC_NS
    nc = _build_program()
    in_maps = _prep_inputs(input, weight)
    res = run_bass_kernel_spmd(nc, in_maps, list(range(NCORES)), trace=TRACE)
    LAST_EXEC_NS = res.exec_time_ns

    vals = np.concatenate(
        [
            np.asarray(res.results[q]["out"], dtype=np.float32).reshape(K, RPC, 64)
            for q in range(NCORES)
        ],
        axis=1,
    )  # (K, 64, 64)
    out = np.zeros((B, K, 128, 128), np.float32)
    out[:, :, ::2, ::2] = vals[None]
    return out


# revision 21
# speedup vs baseline: 1.1309x; 1.1309x over previous
"""Trainium2 Bass kernel for nn_Conv2d_71476845740806.

Reference semantics (buggy naive Conv2d):
  xsum = pad(input, 1).sum(batch)                  # (1, C, 258, 258)
  conv = conv2d(xsum, weight, stride=2, VALID)     # (1, K, 128, 128)
  vals = conv[0, :, :64, :64]                      # (K, 64, 64)
  out  = zeros(B, K, 128, 128); out[:, :, ::2, ::2] = vals  (batch-replicated)

Only window starts (2i, 2j), i,j in [0,64) are used -> only padded rows/cols
0..128 of the summed image matter -> only input rows/cols 0..127.

Device strategy (8 cores, SPMD), v2:
  - Shard the 64 output rows: core q computes rows 8q..8q+7 for ALL K=128
    filters. Needs padded rows 16q..16q+16 (17 rows) x 129 cols, all b, c.
  - bf16 wire format end to end (tolerance is 2e-2; bf16 costs ~0.3% rel
    err): halves HBM traffic and doubles PE/DVE throughput vs fp32/f32r.
  - Host packs per-core xc[128, 8, 1170] bf16:
    partitions 0..63 = (c, even padded rows 0..8), 64..127 = (c, odd rows).
    Each 130-col row is stored even/odd-column-split: [65 even | 65 odd],
    so every conv matmul reads a UNIT-STRIDE 64-col run (the baseline's
    stride-2 rhs read halved PE throughput).
  - Raw hand-synchronized program (no TileContext): the tile epilogue's
    semaphore-reset storm costs ~8us on its own.
  - Batch sum: 4 DVE pair adds as chunks land + 2 quad adds; conv as
    2 groups x 6 matmuls (3 contract-128 for kernel rows 0/1 paired
    across partition halves, 3 contract-64 for row 2) accumulating in
    one PSUM bank [128, 512].
  - Weights ride the scalar-engine HWDGE queue in parallel with input
    chunks on the sync-engine queue; DVE copies PSUM->SBUF; sync DMAs
    the per-core vals (128, 512) out. Host scatters into the zero output.
"""

import contextlib

import ml_dtypes
import numpy as np

import concourse.bacc as bacc
import concourse.mybir as mybir
from concourse.bass_utils import run_bass_kernel_spmd

F32 = mybir.dt.float32
BF16 = mybir.dt.bfloat16

B, C, H, W = 8, 64, 256, 256
K = 128
NCORES = 8
RPC = 8                    # output rows per core (64 total)
ER = 9                     # even padded rows per core
OR = 8                     # odd padded rows per core
WC = 130                   # stored cols per row: [65 even | 65 odd]
PITCH = ER * WC            # 1170 per-batch free pitch
NOUT = RPC * 64            # 512
COLOFF = (0, 65, 1)        # col block offset by kernel-dw (even j / odd j / even j+1)

# Input DMA chunks (must divide 8): 2 = 4-batch chunks (9.4KB bf16 runs),
# 4 = 2-batch chunks (4.7KB runs), 8 = single batches. 4 measured fastest
# (one queued stream of 599KB chunks sustains ~367GB/s after ramp; 1.2MB
# chunks only reach ~220GB/s).
NCHUNK = 4
# Alternate input chunks across the two HWDGE queues (sync + scalar).
SPLITQ = False
# DMA the output as bf16 (host upcasts); halves the tail DMA.
OUT_BF16 = True
# PE p-state warmup: the PE clock ramps with continuous execution (observed
# ~0.8 -> ~1.35 GHz; idle decays it back). Dummy matmuls on (uninitialized)
# SBUF into a scratch PSUM bank keep it busy from t=0, bridge the gap
# between the real groups, and keep the PE sequencer warm into the fixed
# end-of-kernel semaphore-clear ucode (whose issue rate tracks the clock).
WARM_COLS = 256            # rhs free size of a dummy matmul
WARM1 = 30                 # dummies before the first real group
WARM2 = 2                  # dummies between real groups
WARM3 = 0                  # dummies after the last real group (racy: see dummy_mm)
# Wait for the output-DMA completion semaphore before ending the kernel.
# Skipping it (False) lets the end-of-kernel semaphore-clear ucode overlap
# the out-DMA, but the NEFF can then signal done while the out-DMA is still
# in flight on a slow run -- the host reads stale output DRAM (observed as
# sparse wrong cachelines / NaN / wholesale garbage). Keep the wait.
NOWAIT_ODMA = False

# Cap the semaphore space the backend allocates (and clears one-by-one in
# the fixed end-of-kernel ucode, ~115ns/semaphore on the PE sequencer).
# None = backend default. Bass kernel semaphores live at 150+, so keep >=170.
WALRUS_MAX_SEM = None

TRACE = False
LAST_EXEC_NS = None

_cache = {}


def _patch_walrus_args():
    import concourse.bass_utils as bu

    if getattr(bu, "_orig_get_walrus_args", None) is None:
        bu._orig_get_walrus_args = bu.get_walrus_args

        def patched(*a, **kw):
            extra = (
                [f"--max-sem-num={WALRUS_MAX_SEM}"] if WALRUS_MAX_SEM is not None else []
            )
            return bu._orig_get_walrus_args(*a, **kw) + extra

        bu.get_walrus_args = patched


def _build_program():
    key = (NCHUNK, SPLITQ, OUT_BF16, WARM_COLS, WARM1, WARM2, WARM3, NOWAIT_ODMA)
    if key in _cache:
        return _cache[key]

    out_dt = BF16 if OUT_BF16 else F32
    bpc = B // NCHUNK  # batches per DMA chunk

    nc = bacc.Bacc(None)
    xc = nc.declare_dram_parameter("xc", [128, B, PITCH], BF16, isOutput=False)
    # cols 0:384 = pair weights (kernel rows 0/1 on partition halves),
    # cols 384:768 (partitions 0:64) = kernel-row-2 weights.
    wc = nc.declare_dram_parameter("wc", [128, 6 * K], BF16, isOutput=False)
    out = nc.declare_dram_parameter("out", [K, NOUT], out_dt, isOutput=True)

    ctx = contextlib.ExitStack()
    wt = ctx.enter_context(nc.sbuf_tensor([128, 6 * K], BF16))
    staging = ctx.enter_context(nc.sbuf_tensor([128, B * PITCH], BF16))
    spair = ctx.enter_context(nc.sbuf_tensor([128, 4 * PITCH], BF16))
    squad = ctx.enter_context(nc.sbuf_tensor([128, 2 * PITCH], BF16))
    outs = ctx.enter_context(nc.sbuf_tensor([K, NOUT], out_dt))
    # Dedicated scratch for warmup matmuls: reading staging while the input
    # DMA writes it perturbs the DMA (sparse input corruption observed);
    # this tile has no other readers or writers.
    wscr = ctx.enter_context(nc.sbuf_tensor([128, 128 + WARM_COLS], BF16))
    psum = ctx.enter_context(nc.psum_tensor([K, NOUT], F32))
    dpsum = ctx.enter_context(nc.psum_tensor([K, NOUT], F32))
    in_semA = ctx.enter_context(nc.semaphore("in_semA"))
    in_semB = ctx.enter_context(nc.semaphore("in_semB"))
    w_sem = ctx.enter_context(nc.semaphore("w_sem"))
    qadd_sem = ctx.enter_context(nc.semaphore("qadd_sem"))
    mm_sem = ctx.enter_context(nc.semaphore("mm_sem"))
    cp_sem = ctx.enter_context(nc.semaphore("cp_sem"))
    odma_sem = ctx.enter_context(nc.semaphore("odma_sem"))

    wpt3 = wt[:, 0 : 3 * K].rearrange("p (a b) -> p a b", a=3)
    w2t3 = wt[:, 3 * K : 6 * K].rearrange("p (a b) -> p a b", a=3)
    st3 = staging[:, :].rearrange("p (b x) -> p b x", b=B)
    sp3 = spair[:, :].rearrange("p (b x) -> p b x", b=4)
    sq3 = squad[:, :].rearrange("p (b x) -> p b x", b=2)
    psum3 = psum[:, :].rearrange("p (r w) -> p r w", r=RPC)
    xcr = xc[:, :, :]

    def chunk_queue(ch):
        return (ch % 2) if SPLITQ else 0

    def waits_for_chunks(n):
        """(semA_count, semB_count) needed for chunks 0..n-1 complete."""
        if not SPLITQ:
            return n * 16, 0
        a = sum(1 for c in range(n) if chunk_queue(c) == 0)
        b = n - a
        return a * 16, b * 16

    with nc.Block(no_gpsimd_drain=True) as block:

        @block.sync
        def _(sync):
            for ch in range(NCHUNK):
                if chunk_queue(ch) != 0:
                    continue
                b0 = ch * bpc
                sync.dma_start(
                    out=st3[:, b0 : b0 + bpc, :], in_=xcr[:, b0 : b0 + bpc, :]
                ).then_inc(in_semA, 16)
            sync.wait_ge(cp_sem, 1)
            sync.dma_start(out=out[:, :], in_=outs[:, :]).then_inc(odma_sem, 16)
            if not NOWAIT_ODMA:
                sync.wait_ge(odma_sem, 16)

        @block.scalar
        def _(sc):
            sc.dma_start(out=wt[:, :], in_=wc[:, :]).then_inc(w_sem, 16)
            if SPLITQ:
                for ch in range(NCHUNK):
                    if chunk_queue(ch) != 1:
                        continue
                    b0 = ch * bpc
                    sc.dma_start(
                        out=st3[:, b0 : b0 + bpc, :], in_=xcr[:, b0 : b0 + bpc, :]
                    ).then_inc(in_semB, 16)

        @block.vector
        def _(v):
            seenA = seenB = 0
            for bp in range(4):
                nchunks = (2 * bp + 1) // bpc + 1  # chunks needed for batches 2bp,2bp+1
                needA, needB = waits_for_chunks(nchunks)
                if needA > seenA:
                    v.wait_ge(in_semA, needA)
                    seenA = needA
                if needB > seenB:
                    v.wait_ge(in_semB, needB)
                    seenB = needB
                v.tensor_add(sp3[:, bp, :], st3[:, 2 * bp, :], st3[:, 2 * bp + 1, :])
                if bp % 2 == 1:
                    h = bp // 2
                    v.tensor_add(
                        sq3[:, h, :], sp3[:, 2 * h, :], sp3[:, 2 * h + 1, :]
                    ).then_inc(qadd_sem, 1)
            v.wait_ge(mm_sem, 1)
            v.tensor_copy(outs[:, :], psum[:, :]).then_inc(cp_sem, 1)

        @block.tensor
        def _(t):
            def dummy_mm():
                nc.tensor.matmul(
                    dpsum[:, 0:WARM_COLS],
                    wscr[:, 0:128],
                    wscr[:, 128 : 128 + WARM_COLS],
                    start=True,
                    stop=True,
                    skip_group_check=True,
                )

            for _ in range(WARM1):
                dummy_mm()
            t.wait_ge(w_sem, 16)
            for h in range(2):
                t.wait_ge(qadd_sem, h + 1)
                src3 = sq3[:, h, :].rearrange("p (r w) -> p r w", r=ER)
                for dw in range(3):
                    # kernel rows 0 (even rows, row i) + 1 (odd rows, row i)
                    off = COLOFF[dw]
                    nc.tensor.matmul(
                        psum3[:, :, :],
                        wpt3[:, dw, :],
                        src3[:, 0:RPC, off : off + 64],
                        start=(h == 0 and dw == 0),
                        stop=False,
                    )
                for dw in range(3):
                    # kernel row 2: even rows, row i+1
                    off = COLOFF[dw]
                    nc.tensor.matmul(
                        psum3[0:K, :, :],
                        w2t3[0:64, dw, :],
                        src3[0:64, 1 : 1 + RPC, off : off + 64],
                        start=False,
                        stop=(h == 1 and dw == 2),
                    )
                if h == 0:
                    for _ in range(WARM2):
                        dummy_mm()
            # Flush the PE pipeline so the PSUM accumulation is visible
            # before the DVE copy (matmul retire does not imply PSUM
            # writeback -- racing this was the old raw variant's bug).
            # fusable=False keeps the compiler from hoisting the drain above
            # the preceding matmuls; the timed NOP adds writeback margin.
            t.drain(fusable=False)
            t.nop(cycle_cnt=256, nofuse=True)
            t.sem_inc(mm_sem, 1)
            for _ in range(WARM3):
                dummy_mm()

    nc.compile()
    ctx.close()
    _cache[key] = nc
    return nc


def _prep_inputs(input, weight):
    inp = np.ascontiguousarray(input, dtype=np.float32)
    w = np.ascontiguousarray(weight, dtype=np.float32)

    # Padded top-left region: P[r, w] = padded coord (orig r-1, w-1);
    # only padded rows/cols 0..128 are ever read (row/col 129 stays zero).
    P = np.zeros((B, C, 130, 130), np.float32)
    P[:, :, 1:129, 1:129] = inp[:, :, :128, :128]
    # Even/odd column split: row -> [65 even cols | 65 odd cols], so the
    # device matmuls read unit-stride 64-col runs.
    R = np.concatenate([P[:, :, :, 0:130:2], P[:, :, :, 1:130:2]], axis=3)
    Rc = np.ascontiguousarray(R.transpose(1, 0, 2, 3))  # (C, B, 130, 130)

    t = [w[:, :, dh, :].transpose(1, 2, 0).reshape(-1, 3 * K) for dh in range(3)]
    wc_host = np.zeros((128, 6 * K), np.float32)
    wc_host[0:64, 0 : 3 * K] = t[0]
    wc_host[64:128, 0 : 3 * K] = t[1]
    wc_host[0:64, 3 * K : 6 * K] = t[2]
    wc_host = np.ascontiguousarray(wc_host.astype(ml_dtypes.bfloat16))

    in_maps = []
    for q in range(NCORES):
        r0 = 16 * q
        xcq = np.zeros((128, B, PITCH), np.float32)
        xcq[0:64] = Rc[:, :, r0 : r0 + 17 : 2, :].reshape(64, B, PITCH)
        xcq[64:128, :, 0 : OR * WC] = Rc[:, :, r0 + 1 : r0 + 16 : 2, :].reshape(
            64, B, OR * WC
        )
        in_maps.append(
            {"xc": np.ascontiguousarray(xcq.astype(ml_dtypes.bfloat16)), "wc": wc_host}
        )
    return in_maps


def kernel(input, weight):
    global LAST_EXEC_NS
    _patch_walrus_args()
    nc = _build_program()
    in_maps = _prep_inputs(input, weight)
    res = run_bass_kernel_spmd(nc, in_maps, list(range(NCORES)), trace=TRACE)
    LAST_EXEC_NS = res.exec_time_ns

    vals = np.concatenate(
        [
            np.asarray(res.results[q]["out"], dtype=np.float32).reshape(K, RPC, 64)
            for q in range(NCORES)
        ],
        axis=1,
    )  # (K, 64, 64)
    out = np.zeros((B, K, 128, 128), np.float32)
    out[:, :, ::2, ::2] = vals[None]
    return out
